# revision 23
# baseline (speedup 1.0000x reference)
"""Trainium2 Bass kernel for nn_DSModelMultiQ (Dempster-Shafer rule model).

Math (per batch sample):
  xg = X[:, lit_feat_idx]                      gather      [B, L]
  truth = op-dependent compare(xg, lit_value)  elementwise [B, L]
  fired = (truth @ lit2rule >= rule_len - .5)  == every rule is a conjunction
          of exactly 3 literal slots -> fired = AND of its 3 truth rows
  masses = softmax(rule_mass_params)           [R, K+1]
  q/w = exp(fired @ [log(m_k+om) | log(om)])
  out  = (relu(q-w) + w*prior) / max(sum(relu(q-w)) + w, eps)

v4 sharding: 4-way batch x 2-way rules over the 8 NeuronCores.  Core cid
handles batch slice cid//2 (2048 rows) and rule half cid%2 (4096 rules).
Dempster combination is a product over rules == sum in log space, so the
two partner cores AllReduce(add) their partial fired@log-mass GEMM results
(a [66, 2048] f32 tile) and both run the identical epilogue.  Rule
sharding halves the SWDGE descriptor count (the Q7 descriptor-generation
at ~8ns/row is the dominant serial cost: 24576 rows/core batch-only vs
12288 here), while batch 2048 keeps the truth-table compute/write and the
gather payload rows (2KB) efficient.

Other key design points:
  - truth table in DRAM as fp8(e4m3) {0,1}; compares write f8 directly.
  - chunk families: '<'/'>' literals live in is_le-form chunks
    (truth = ((x <= s1) == s2)), '==' literals in is_equal-form chunks
    (truth = (x == s1) * s2).  Equality is exact, no fallback pass.
  - fired gathers are dma_gather(prepare_only)+trigger_dma; the gather
    source AP is sliced to a per-gather row prefix and rules are sorted by
    their max literal row, so early triggers only depend on early truth
    chunk writes (descriptor generation overlaps the truth phase).
  - explicit nc.vector.wait_ge on the DMA completion semaphores (Tile's
    automatic RAW edge from a prepared gather to its consumer is racy).
  - the 3-literal AND: 2 bitwise_and ops on int16 views of f8 rows (DVE 2x
    packed mode); fired stays f8 and feeds the matmul directly against the
    f16 stationary [delta | base_lo | base_hi] (mixed dtype is exact).
  - mass reformulation: log(m_k+om) = base + delta_k with base = log(om)
    split hi+lo f16 and delta_k = log1p(e_k/e_om) f16: one stationary
    [128, 66] per rule chunk, fired is the moving operand.
"""

import numpy as np
import ml_dtypes  # noqa: F401

from concourse import bacc
import concourse.bass as bass
import concourse.mybir as mybir
import concourse.tile as tile
from concourse.bass_utils import run_bass_kernel_spmd
from concourse.masks import make_identity
from concourse import library_config

F32 = mybir.dt.float32
F16 = mybir.dt.float16
F8 = mybir.dt.float8e4
I32 = mybir.dt.int32
I16 = mybir.dt.int16
AF = mybir.ActivationFunctionType
OP = mybir.AluOpType
AX = mybir.AxisListType

EPS = 1e-12

B, F, L, R, K = 8192, 128, 4096, 8192, 64
N_CORES = 8
BS = 4                 # batch shards
RS = 2                 # rule shards
BL = B // BS           # 2048 batch rows per core
RL = R // RS           # 4096 rules per core
KP = K + 1
W = K + 2              # stationary width: delta(64) | base_lo | base_hi
XG_G = 2               # overflow chunks per x-gather instruction
FG = 2                 # rule chunks per fired-gather instruction
NH = BL // 512         # PSUM banks for the mass GEMM (4)
NB = BL // 128         # output row chunks (16)
FW_COLS = 2 * FG * 128 // 16   # two gathered streams per anchored chunk


def build_nc4(layout, stage=5):
    n_cmp, n_eq, b_cmp, b_eq, n_extra, prefixes = layout
    TC = n_cmp + n_eq + b_cmp + b_eq
    RCC = TC + n_extra          # anchored chunks + overflow chunks
    b_chunks = b_cmp + b_eq
    fam = [0] * n_cmp + [1] * n_eq + [0] * b_cmp + [1] * b_eq

    nc = bacc.Bacc(None, target_bir_lowering=False, num_swdge_queues=2,
                   dynamic_dma_scratch_size=32768)

    xT = nc.dram_tensor("xT", [F, BL], F32, kind="ExternalInput")
    s1 = nc.dram_tensor("s1", [128, TC], F32, kind="ExternalInput")
    s2 = nc.dram_tensor("s2", [128, TC], F32, kind="ExternalInput")
    bidx = nc.dram_tensor("bidx", [128, max(b_chunks, 1) * 8], I16,
                          kind="ExternalInput")
    n_fgcols = ((TC + FG - 1) // FG) * FW_COLS + n_extra * 24
    fgidx = nc.dram_tensor("fgidx", [128, n_fgcols], I16,
                           kind="ExternalInput")
    rmp = nc.dram_tensor("rmp", [128, RCC * KP], F32, kind="ExternalInput")
    prior = nc.dram_tensor("prior", [128, K], F32, kind="ExternalInput")
    out = nc.dram_tensor("out", [BL // 2, K], F32, kind="ExternalOutput")
    ccin = nc.dram_tensor("ccin", [128, NB * W], F32)
    ccout = nc.dram_tensor("ccout", [64, NB * W], F32)

    with tile.TileContext(nc) as tc:
        with (
            tc.tile_pool(name="consts", bufs=1) as cp,
            tc.tile_pool(name="persist", bufs=1) as pp,
            tc.tile_pool(name="dramp", bufs=1, space="DRAM") as dp,
            tc.tile_pool(name="xgp", bufs=2) as xgp,
            tc.tile_pool(name="truthp", bufs=2) as trp,
            tc.tile_pool(name="ggp", bufs=5) as ggp,
            tc.tile_pool(name="psacc", bufs=1, space="PSUM") as pacc,
            tc.tile_pool(name="pstr", bufs=2, space="PSUM") as ptr,
            tc.tile_pool(name="ep", bufs=1) as ep,
        ):
            # ---- constant loads (HWDGE; index tables first, queues split) ----
            bidx_sb = cp.tile([128, max(b_chunks, 1) * 8], I16)
            nc.scalar.dma_start(bidx_sb[:], bidx.ap())
            fgidx_sb = cp.tile([128, n_fgcols], I16)
            nc.scalar.dma_start(fgidx_sb[:], fgidx.ap())
            s1_sb = cp.tile([128, TC], F32)
            nc.sync.dma_start(s1_sb[:], s1.ap())
            s2_sb = cp.tile([128, TC], F32)
            nc.sync.dma_start(s2_sb[:], s2.ap())
            xT_sb = cp.tile([128, BL], F32)
            nc.sync.dma_start(xT_sb[:], xT.ap())
            prior_sb = cp.tile([128, K], F32)
            nc.scalar.dma_start(prior_sb[:], prior.ap())
            ident = cp.tile([128, 128], F32)
            make_identity(nc, ident[:])
            nc.gpsimd.load_library(library_config.mlp)

            # ---- prep: per-rule [delta | base_lo | base_hi] f16, wide ----
            rhs = pp.tile([128, RCC, W], F16)  # stationary per chunk
            with (
                tc.tile_pool(name="prepbig", bufs=1) as pb,
                tc.tile_pool(name="prepsm", bufs=1) as ps,
            ):
                rmp_sb = pb.tile([128, RCC * KP], F32)
                nc.scalar.dma_start(rmp_sb[:], rmp.ap())
                rmp3 = rmp_sb[:].rearrange("p (c k) -> p c k", k=KP)
                rmp_om = ps.tile([128, RCC], F32)
                nc.vector.tensor_copy(rmp_om[:], rmp3[:, :, K])
                nc.scalar.activation(rmp_sb[:], rmp_sb[:], AF.Exp)
                e3 = rmp3
                z = ps.tile([128, RCC], F32)
                nc.vector.tensor_reduce(z[:], e3, AX.X, OP.add)
                lnz = ps.tile([128, RCC], F32)
                nc.scalar.activation(lnz[:], z[:], AF.Ln)
                base = ps.tile([128, RCC], F32)
                nc.vector.tensor_tensor(base[:], rmp_om[:], lnz[:], OP.subtract)
                bh = ps.tile([128, RCC], F16)
                nc.vector.tensor_copy(bh[:], base[:])
                nc.vector.tensor_tensor(rhs[:, :, K], base[:], bh[:], OP.subtract)
                nc.vector.tensor_copy(rhs[:, :, K + 1], bh[:])
                rec = ps.tile([128, RCC], F32)
                nc.vector.reciprocal(rec[:], e3[:, :, K])
                r3 = e3[:, :, 0:K]
                nc.vector.tensor_tensor(
                    r3, r3, rec[:].unsqueeze(2).to_broadcast([128, RCC, K]),
                    OP.mult)
                nc.scalar.activation(rhs[:, :, 0:K], r3, AF.Ln, bias=1.0)

            # ---- truth table -> DRAM (fp8) ----
            truth_dram = dp.tile([TC * 128, BL], F8)
            td = truth_dram[:].rearrange("(c p) b -> p c b", p=128)

            def compare(dst, src, c):
                if fam[c] == 0:
                    nc.vector.tensor_scalar(
                        dst, src, s1_sb[:, c:c + 1], s2_sb[:, c:c + 1],
                        OP.is_le, op1=OP.is_equal)
                else:
                    nc.vector.tensor_scalar(
                        dst, src, s1_sb[:, c:c + 1], s2_sb[:, c:c + 1],
                        OP.is_equal, op1=OP.mult)

            xg_sem = [nc.alloc_semaphore("xg_dma0"),
                      nc.alloc_semaphore("xg_dma1")]
            xg_count = [0, 0]
            n_xg = (b_chunks + XG_G - 1) // XG_G if stage >= 2 else 0

            def xg_prep(g):
                nch = min(XG_G, b_chunks - g * XG_G)
                xg = xgp.tile([128, XG_G, BL], F32, name="xgB")
                qn = g % 2
                nc.gpsimd.dma_gather(
                    xg[:, 0:nch, :], xT.ap(),
                    bidx_sb[:, g * XG_G * 8:g * XG_G * 8 + nch * 8],
                    nch * 128, nch * 128, BL,
                    prepare_only=True, sem=xg_sem[qn], queue_num=qn,
                )
                nc.gpsimd.trigger_dma(count=None, queue_num=qn)
                xg_count[qn] += 1
                return xg, nch, qn, xg_count[qn]

            xg_pend = [xg_prep(g) for g in range(min(2, n_xg))]
            n_ab = n_cmp + n_eq
            SG = 2
            for g in range((n_ab + SG - 1) // SG if stage >= 2 else 0):
                nch = min(SG, n_ab - g * SG)
                tr = trp.tile([128, SG, BL], F8)
                for i in range(nch):
                    compare(tr[:, i, :], xT_sb[:], g * SG + i)
                eng = nc.sync if g % 2 == 0 else nc.scalar
                eng.dma_start(td[:, g * SG:g * SG + nch, :],
                              tr[:, 0:nch, :])
            for g in range(n_xg):
                xg, nch, qn, cnt = xg_pend[g]
                if g + 2 < n_xg:
                    xg_pend.append(xg_prep(g + 2))
                nc.vector.wait_ge(xg_sem[qn], 16 * cnt)
                tr = trp.tile([128, XG_G, BL], F8)
                for i in range(nch):
                    compare(tr[:, i, :], xg[:, i, :], n_ab + g * XG_G + i)
                eng = nc.sync if g % 2 == 0 else nc.scalar
                eng.dma_start(
                    td[:, n_ab + g * XG_G:n_ab + g * XG_G + nch, :],
                    tr[:, 0:nch, :])

            # ---- fired + mass matmul ----
            # Rule slot s == truth table row s: the anchor literal of the
            # rule at slot s IS row s, so the anchor stream is a sequential
            # HWDGE read of the table (no SWDGE descriptors).  Only the two
            # non-anchor literals are gathered (2/3 of the descriptors).
            # Unmatched rules live in `n_extra` trailing chunks with all
            # three literals gathered.
            q_ps = []
            for h in range(NH):
                t = pacc.tile([W, 512], F32, name=f"qps{h}")
                nc.vector.memset(t[:], 0.0)
                q_ps.append(t)

            RCC_T = TC                  # anchored rule chunks
            n_gath = (RCC_T + FG - 1) // FG
            last_rc = RCC_T + n_extra - 1
            fg_sem = [nc.alloc_semaphore("fg_dma0"),
                      nc.alloc_semaphore("fg_dma1")]
            fg_count = [0, 0]
            truth_i16 = truth_dram[:].bitcast(I16)
            for fg in range(n_gath if stage >= 3 else 0):
                qn = (fg // 2) % 2
                nch = min(FG, RCC_T - fg * FG)
                pfx = TC * 128 if prefixes is None else prefixes[fg]
                gg = ggp.tile([128, 2 * FG, BL // 2], I16, name="gg")
                nc.gpsimd.dma_gather(
                    gg[:, 0:2 * nch, :], truth_dram[0:pfx, :].bitcast(I16),
                    fgidx_sb[:, fg * FW_COLS:fg * FW_COLS + 2 * nch * 8],
                    2 * nch * 128, 2 * nch * 128, BL // 2,
                    prepare_only=True, sem=fg_sem[qn], queue_num=qn,
                )
                nc.gpsimd.trigger_dma(count=None, queue_num=qn)
                fg_count[qn] += 1
                # sequential anchor block = the truth chunks themselves
                tseq = ggp.tile([128, FG, BL // 2], I16, name="tseq")
                nc.scalar.dma_start(
                    tseq[:, 0:nch, :],
                    truth_i16[fg * FG * 128:(fg * FG + nch) * 128, :]
                    .rearrange("(c p) b -> p c b", p=128))
                nc.vector.wait_ge(fg_sem[qn], 16 * fg_count[qn])
                nc.vector.tensor_tensor(
                    gg[:, 0:nch, :], gg[:, 0:nch, :],
                    gg[:, nch:2 * nch, :], OP.bitwise_and)
                nc.vector.tensor_tensor(
                    gg[:, 0:nch, :], gg[:, 0:nch, :], tseq[:, 0:nch, :],
                    OP.bitwise_and)
                fired = gg[:, 0:FG, :].bitcast(F8)  # [128, FG, BL]
                for i in range(nch if stage >= 4 else 0):
                    rc = fg * FG + i
                    for h in range(NH):
                        nc.tensor.matmul(
                            q_ps[h][:],
                            lhsT=rhs[:, rc, :],
                            rhs=fired[:, i, h * 512:(h + 1) * 512],
                            start=False, stop=(rc == last_rc),
                            skip_group_check=True,
                        )
            # extra chunks: fully gathered (3 streams)
            for e in range(n_extra if stage >= 3 else 0):
                qn = e % 2
                base = n_gath * FW_COLS + e * 24
                gg = ggp.tile([128, 3, BL // 2], I16, name="gge")
                nc.gpsimd.dma_gather(
                    gg[:], truth_dram[:].bitcast(I16),
                    fgidx_sb[:, base:base + 24],
                    3 * 128, 3 * 128, BL // 2,
                    prepare_only=True, sem=fg_sem[qn], queue_num=qn,
                )
                nc.gpsimd.trigger_dma(count=None, queue_num=qn)
                fg_count[qn] += 1
                nc.vector.wait_ge(fg_sem[qn], 16 * fg_count[qn])
                nc.vector.tensor_tensor(
                    gg[:, 0:1, :], gg[:, 0:1, :], gg[:, 1:2, :],
                    OP.bitwise_and)
                nc.vector.tensor_tensor(
                    gg[:, 0:1, :], gg[:, 0:1, :], gg[:, 2:3, :],
                    OP.bitwise_and)
                fired = gg[:, 0:1, :].bitcast(F8)
                rc = RCC_T + e
                for h in range(NH):
                    nc.tensor.matmul(
                        q_ps[h][:],
                        lhsT=rhs[:, rc, :],
                        rhs=fired[:, 0, h * 512:(h + 1) * 512],
                        start=False, stop=(rc == last_rc),
                        skip_group_check=True,
                    )

            # ---- transpose partials, pairwise log-space ReduceScatter ----
            qlog = ep.tile([W, BL], F32, name="qlog")
            for h in range(NH):
                nc.vector.tensor_copy(qlog[:, h * 512:(h + 1) * 512],
                                      q_ps[h][:])
            widef = ep.tile([128, NB, W], F32, name="widef")
            for g in range(NB):
                tp = ptr.tile([128, W], F32)
                nc.tensor.transpose(
                    tp[:], qlog[:, g * 128:(g + 1) * 128], ident[0:W, 0:W])
                nc.vector.tensor_copy(widef[:, g, :], tp[:])
            nc.sync.dma_start(ccin.ap(), widef[:].rearrange("p g w -> p (g w)"))
            nc.gpsimd.collective_compute(
                "ReduceScatter", OP.add,
                replica_groups=[[2 * i, 2 * i + 1] for i in range(BS)],
                ins=[ccin.ap().opt()], outs=[ccout.ap().opt()],
            )
            wide = ep.tile([64, NB, W], F32, name="wide")
            nc.scalar.dma_start(wide[:].rearrange("p g w -> p (g w)"),
                                ccout.ap())

            # ---- epilogue (wide, half rows per core) ----
            logw = ep.tile([64, NB], F32, name="logw")
            nc.vector.tensor_tensor(
                logw[:], wide[:, :, K], wide[:, :, K + 1], OP.add)
            logq = ep.tile([64, NB, K], F32, name="logq")
            nc.vector.tensor_tensor(
                logq[:], wide[:, :, 0:K],
                logw[:].unsqueeze(2).to_broadcast([64, NB, K]), OP.add)
            q = logq
            nc.scalar.activation(q[:], logq[:], AF.Exp)
            wv = ep.tile([64, NB], F32, name="wv")
            nc.scalar.activation(wv[:], logw[:], AF.Exp)
            belief = q
            nc.vector.tensor_tensor(
                belief[:], q[:],
                wv[:].unsqueeze(2).to_broadcast([64, NB, K]), OP.subtract)
            nc.vector.tensor_scalar(belief[:], belief[:], 0.0, None, OP.max)
            bsum = ep.tile([64, NB], F32, name="bsum")
            nc.vector.tensor_reduce(bsum[:], belief[:], AX.X, OP.add)
            tsum = ep.tile([64, NB], F32, name="tsum")
            nc.vector.tensor_tensor(tsum[:], bsum[:], wv[:], OP.add)
            total = ep.tile([64, NB], F32, name="total")
            nc.vector.tensor_scalar(total[:], tsum[:], EPS, None, OP.max)
            rtot = ep.tile([64, NB], F32, name="rtot")
            nc.vector.reciprocal(rtot[:], total[:])
            wp = ep.tile([64, NB, K], F32, name="wp")
            nc.vector.tensor_tensor(
                wp[:], wv[:].unsqueeze(2).to_broadcast([64, NB, K]),
                prior_sb[0:64, :].unsqueeze(1).to_broadcast([64, NB, K]),
                OP.mult)
            nc.vector.tensor_tensor(belief[:], belief[:], wp[:], OP.add)
            nc.vector.tensor_tensor(
                belief[:], belief[:],
                rtot[:].unsqueeze(2).to_broadcast([64, NB, K]), OP.mult)
            nc.sync.dma_start(
                out.ap().rearrange("(g p) k -> p g k", p=64), belief[:])

    return nc


def _family_chunks(counts, cap_overflow=256):
    """Smallest per-feature chunk count n with total overflow <= cap."""
    hi = int(counts.max()) if counts.size else 0
    n = hi
    while n > 0 and int(np.maximum(counts - (n - 1), 0).sum()) <= cap_overflow:
        n -= 1
    return n


def host_layout(lit_feat_idx, lit_op_code):
    """Chunk-family layout: per-literal renumbering into the padded table."""
    fidx = np.asarray(lit_feat_idx, dtype=np.int64)
    op = np.asarray(lit_op_code)
    fam_lit = (op == 0).astype(np.int64)  # 1 for '=='
    cnt = np.zeros((2, F), np.int64)
    for fm in (0, 1):
        cnt[fm] = np.bincount(fidx[fam_lit == fm], minlength=F)
    n_cmp = _family_chunks(cnt[0])
    n_eq = _family_chunks(cnt[1])
    ov_cmp = int(np.maximum(cnt[0] - n_cmp, 0).sum())
    ov_eq = int(np.maximum(cnt[1] - n_eq, 0).sum())
    b_cmp = (ov_cmp + 127) // 128
    b_eq = (ov_eq + 127) // 128
    return n_cmp, n_eq, b_cmp, b_eq


def host_prep(X, lit_value, lit2rule, rule_len, rule_mass_params, prior,
              lit_feat_idx, lit_op_code):
    """Pure data-marshaling: shard X over batch and rules over parity,
    extract per-rule literal ids from lit2rule, compute per-literal (s1, s2)
    compare scalars for the chunk-family layout, pre-transpose rmp."""
    X = np.asarray(X, dtype=np.float32)
    lit_value = np.asarray(lit_value, dtype=np.float32)
    lit2rule = np.asarray(lit2rule, dtype=np.float32)
    rule_mass_params = np.asarray(rule_mass_params, dtype=np.float32)
    prior = np.asarray(prior, dtype=np.float32)
    op = np.asarray(lit_op_code)
    fidx = np.asarray(lit_feat_idx, dtype=np.int64)

    lT = lit2rule.T
    r_idx, l_idx = np.nonzero(lT)
    cnt_rl = lT[r_idx, l_idx].astype(np.int64)
    rl = np.repeat(l_idx, cnt_rl)
    assert rl.size == 3 * R, rl.size
    rule_lits = rl.reshape(R, 3).astype(np.int64)

    n_cmp, n_eq, b_cmp, b_eq = host_layout(fidx, op)
    n_ab = n_cmp + n_eq
    b_chunks = b_cmp + b_eq
    TC = n_ab + b_chunks
    LP = TC * 128

    # per-literal compare scalars in the family forms
    pred_v = np.nextafter(lit_value, -np.inf)
    s1 = np.where(op == 0, lit_value,
                  np.where(op == 1, pred_v, lit_value)).astype(np.float32)
    s2 = np.where(op == 0, 1.0,
                  np.where(op == 1, 1.0, 0.0)).astype(np.float32)

    # pad defaults per chunk family (always-false)
    fam = [0] * n_cmp + [1] * n_eq + [0] * b_cmp + [1] * b_eq
    s1n = np.empty(LP, np.float32)
    s2n = np.empty(LP, np.float32)
    for c in range(TC):
        if fam[c] == 0:
            s1n[c * 128:(c + 1) * 128] = -1.0
            s2n[c * 128:(c + 1) * 128] = 3.0
        else:
            s1n[c * 128:(c + 1) * 128] = -1.0
            s2n[c * 128:(c + 1) * 128] = 0.0

    newid = np.empty(L, np.int64)
    bfeat = np.zeros(max(b_chunks, 1) * 128, np.int16)
    ovbase = {0: 0, 1: b_cmp * 128}
    ovcount = [0, 0]
    fambase = {0: 0, 1: n_cmp}
    famcap = {0: n_cmp, 1: n_eq}
    for f in range(F):
        for fm in (0, 1):
            lits = np.nonzero((fidx == f) & ((op == 0) == bool(fm)))[0]
            for c, l in enumerate(lits):
                if c < famcap[fm]:
                    nid = (fambase[fm] + c) * 128 + f
                else:
                    slot = ovbase[fm] + ovcount[fm]
                    nid = (n_ab * 128) + slot
                    bfeat[slot] = f
                    ovcount[fm] += 1
                newid[l] = nid
                s1n[nid] = s1[l]
                s2n[nid] = s2[l]
    assert ovcount[0] <= b_cmp * 128 and ovcount[1] <= b_eq * 128
    rule_rows = newid[rule_lits]  # [R, 3] table rows

    def col128(v):
        return np.ascontiguousarray(np.asarray(v).reshape(-1, 128).T)

    def wrap16(ids):
        ids = np.asarray(ids, dtype=np.int16)
        return np.tile(ids.reshape(-1, 16).T, (8, 1))

    # bidx blocks are per x-gather instruction: XG_G chunks each, last partial
    bblocks = []
    g = 0
    while g * XG_G < b_chunks:
        nch = min(XG_G, b_chunks - g * XG_G)
        bblocks.append(wrap16(bfeat[g * XG_G * 128:(g * XG_G + nch) * 128]))
        g += 1
    if not bblocks:
        bblocks = [wrap16(bfeat)]
    shared = {
        "s1": col128(s1n), "s2": col128(s2n),
        "prior": np.ascontiguousarray(
            np.broadcast_to(prior.reshape(1, K), (128, K))),
        "bidx": np.concatenate(bblocks, axis=1),
    }

    # per rule half: match each rule to an anchor literal (slot == table
    # row), build the two gathered streams + overflow chunks in slot order
    import sys
    sys.setrecursionlimit(200000)
    n_slots = TC * 128
    RPAD = np.concatenate([np.full(K, -40.0, np.float32),
                           np.full(1, 40.0, np.float32)])

    def match_anchors(rows_h):
        owner = np.full(n_slots, -1, np.int64)

        def try_assign(r, visited):
            for l in rows_h[r]:
                if owner[l] == -1:
                    owner[l] = r
                    return True
            for l in rows_h[r]:
                li = int(l)
                if li not in visited:
                    visited.add(li)
                    if try_assign(owner[li], visited):
                        owner[li] = r
                        return True
            return False

        leftovers = [r for r in range(rows_h.shape[0])
                     if not try_assign(r, set())]
        return owner, leftovers

    halves = []
    n_extra_max = 0
    for h in range(RS):
        rows_h = rule_rows[h * RL:(h + 1) * RL]
        owner, leftovers = match_anchors(rows_h)
        n_extra_max = max(n_extra_max, (len(leftovers) + 127) // 128)
        halves.append((rows_h, owner, leftovers))
    n_extra = n_extra_max

    built = []
    for h in range(RS):
        rows_h, owner, leftovers = halves[h]
        # two non-anchor rows per slot (pads -> row 0)
        g12 = np.zeros((n_slots, 2), np.int64)
        rmp_rows = np.tile(RPAD, (TC * 128 + n_extra * 128, 1))
        rmp_h = rule_mass_params[h * RL:(h + 1) * RL]
        for s in range(n_slots):
            r = owner[s]
            if r < 0:
                continue
            rl = list(rows_h[r])
            rl.remove(s)
            g12[s] = rl
            rmp_rows[s] = rmp_h[r]
        n_gath = (TC + FG - 1) // FG
        fblocks = []
        prefixes = []
        for fg in range(n_gath):
            c0 = fg * FG
            nch = min(FG, TC - c0)
            ids = np.concatenate(
                [g12[(c0 + c) * 128:(c0 + c + 1) * 128, j]
                 for j in range(2) for c in range(nch)])
            blk16 = wrap16(ids)
            if blk16.shape[1] < FW_COLS:  # pad partial last gather's block
                blk16 = np.concatenate(
                    [blk16, np.zeros((128, FW_COLS - blk16.shape[1]),
                                     np.int16)], axis=1)
            fblocks.append(blk16)
            pfx = int(ids.max()) + 1
            prefixes.append(min((pfx + 511) // 512 * 512, TC * 128))
        # overflow chunks: 3 gathered streams
        ex_rows = np.zeros((n_extra * 128, 3), np.int64)
        for i, r in enumerate(leftovers):
            ex_rows[i] = rows_h[r]
            rmp_rows[n_slots + i] = rmp_h[r]
        for e in range(n_extra):
            blk = ex_rows[e * 128:(e + 1) * 128]
            ids = np.concatenate([blk[:, j] for j in range(3)])
            fblocks.append(wrap16(ids))
        fgidx = np.concatenate(fblocks, axis=1)
        RCC = TC + n_extra
        rmp_wide = np.ascontiguousarray(
            rmp_rows.reshape(RCC, 128, KP).transpose(1, 0, 2).reshape(128, -1))
        built.append((fgidx, rmp_wide, tuple(prefixes)))
    halves = built

    layout = (n_cmp, n_eq, b_cmp, b_eq, n_extra,
              tuple(max(built[0][2][i], built[1][2][i])
                    for i in range(len(built[0][2]))))
    in_maps = []
    for cid in range(N_CORES):
        s, h = cid // RS, cid % RS
        m = dict(shared)
        m["xT"] = np.ascontiguousarray(X[s * BL:(s + 1) * BL, :].T)
        m["fgidx"] = halves[h][0]
        m["rmp"] = halves[h][1]
        in_maps.append(m)
    return in_maps, layout


_NC_CACHE = {}


def build_nc_cached(layout):
    import os
    stage = int(os.environ.get("KSTAGE", "5"))
    key = (layout, stage)
    if key not in _NC_CACHE:
        nc = build_nc4(layout, stage=stage)
        nc.finalize()
        _NC_CACHE[key] = nc
    return _NC_CACHE[key]


def kernel(**inputs) -> np.ndarray:
    in_maps, layout = host_prep(
        inputs["X"], inputs["lit_value"], inputs["lit2rule"],
        inputs["rule_len"], inputs["rule_mass_params"], inputs["prior"],
        inputs["lit_feat_idx"], inputs["lit_op_code"],
    )
    nc = build_nc_cached(layout)
    res = run_bass_kernel_spmd(nc, in_maps, core_ids=list(range(N_CORES)))
    return assemble(res)


def assemble(res):
    """Each core holds rows with (b % 128) in its 64-partition half."""
    full = np.empty((B, K), np.float32)
    for cid in range(N_CORES):
        s, h = cid // RS, cid % RS
        o = res.results[cid]["out"].reshape(NB, 64, K)
        full.reshape(BS, NB, 128, K)[s, :, h * 64:(h + 1) * 64, :] = o
    return full


# revision 31
# speedup vs baseline: 1.0480x; 1.0480x over previous
"""Trainium2 Bass kernel for nn_DSModelMultiQ (Dempster-Shafer rule model).

Math (per batch sample):
  xg = X[:, lit_feat_idx]                      gather      [B, L]
  truth = op-dependent compare(xg, lit_value)  elementwise [B, L]
  fired = (truth @ lit2rule >= rule_len - .5)  == every rule is a conjunction
          of exactly 3 literal slots -> fired = AND of its 3 truth rows
  masses = softmax(rule_mass_params)           [R, K+1]
  q/w = exp(fired @ [log(m_k+om) | log(om)])
  out  = (relu(q-w) + w*prior) / max(sum(relu(q-w)) + w, eps)

v4 sharding: 4-way batch x 2-way rules over the 8 NeuronCores.  Core cid
handles batch slice cid//2 (2048 rows) and rule half cid%2 (4096 rules).
Dempster combination is a product over rules == sum in log space, so the
two partner cores AllReduce(add) their partial fired@log-mass GEMM results
(a [66, 2048] f32 tile) and both run the identical epilogue.  Rule
sharding halves the SWDGE descriptor count (the Q7 descriptor-generation
at ~8ns/row is the dominant serial cost: 24576 rows/core batch-only vs
12288 here), while batch 2048 keeps the truth-table compute/write and the
gather payload rows (2KB) efficient.

Other key design points:
  - truth table in DRAM as fp8(e4m3) {0,1}; compares write f8 directly.
  - chunk families: '<'/'>' literals live in is_le-form chunks
    (truth = ((x <= s1) == s2)), '==' literals in is_equal-form chunks
    (truth = (x == s1) * s2).  Equality is exact, no fallback pass.
  - fired gathers are dma_gather(prepare_only)+trigger_dma; the gather
    source AP is sliced to a per-gather row prefix and rules are sorted by
    their max literal row, so early triggers only depend on early truth
    chunk writes (descriptor generation overlaps the truth phase).
  - explicit nc.vector.wait_ge on the DMA completion semaphores (Tile's
    automatic RAW edge from a prepared gather to its consumer is racy).
  - the 3-literal AND: 2 bitwise_and ops on int16 views of f8 rows (DVE 2x
    packed mode); fired stays f8 and feeds the matmul directly against the
    f16 stationary [delta | base_lo | base_hi] (mixed dtype is exact).
  - mass reformulation: log(m_k+om) = base + delta_k with base = log(om)
    split hi+lo f16 and delta_k = log1p(e_k/e_om) f16: one stationary
    [128, 66] per rule chunk, fired is the moving operand.
"""

import numpy as np
import ml_dtypes  # noqa: F401

from concourse import bacc
import concourse.bass as bass
import concourse.mybir as mybir
import concourse.tile as tile
from concourse.bass_utils import run_bass_kernel_spmd
from concourse.masks import make_identity
from concourse import library_config

F32 = mybir.dt.float32
F16 = mybir.dt.float16
F8 = mybir.dt.float8e4
I32 = mybir.dt.int32
I16 = mybir.dt.int16
AF = mybir.ActivationFunctionType
OP = mybir.AluOpType
AX = mybir.AxisListType

EPS = 1e-12

B, F, L, R, K = 8192, 128, 4096, 8192, 64
N_CORES = 8
BS = 4                 # batch shards
RS = 2                 # rule shards
BL = B // BS           # 2048 batch rows per core
RL = R // RS           # 4096 rules per core
KP = K + 1
W = K + 2              # stationary width: delta(64) | base_lo | base_hi
XG_G = 2               # overflow chunks per x-gather instruction
FG = 3                 # rule chunks per fired-gather instruction
NH = BL // 512         # PSUM banks for the mass GEMM (4)
NB = BL // 128         # output row chunks (16)
FW_COLS = 2 * FG * 128 // 16   # two gathered streams per anchored chunk


def build_nc4(layout, stage=5):
    n_cmp, n_eq, b_cmp, b_eq, n_extra, prefixes = layout
    TC = n_cmp + n_eq + b_cmp + b_eq
    RCC = TC + n_extra          # anchored chunks + overflow chunks
    b_chunks = b_cmp + b_eq
    fam = [0] * n_cmp + [1] * n_eq + [0] * b_cmp + [1] * b_eq

    nc = bacc.Bacc(None, target_bir_lowering=False, num_swdge_queues=2,
                   dynamic_dma_scratch_size=32768)

    xT = nc.dram_tensor("xT", [F, BL], F32, kind="ExternalInput")
    s1 = nc.dram_tensor("s1", [128, TC], F32, kind="ExternalInput")
    s2 = nc.dram_tensor("s2", [128, TC], F32, kind="ExternalInput")
    bidx = nc.dram_tensor("bidx", [128, max(b_chunks, 1) * 8], I16,
                          kind="ExternalInput")
    n_fgcols = ((TC + FG - 1) // FG) * FW_COLS + n_extra * 24
    fgidx = nc.dram_tensor("fgidx", [128, n_fgcols], I16,
                           kind="ExternalInput")
    rmp = nc.dram_tensor("rmp", [128, RCC * KP], F32, kind="ExternalInput")
    prior = nc.dram_tensor("prior", [128, K], F32, kind="ExternalInput")
    out = nc.dram_tensor("out", [BL // 2, K], F32, kind="ExternalOutput")
    ccin = nc.dram_tensor("ccin", [128, NB * W], F32)
    ccout = nc.dram_tensor("ccout", [64, NB * W], F32)

    with tile.TileContext(nc) as tc:
        with (
            tc.tile_pool(name="consts", bufs=1) as cp,
            tc.tile_pool(name="persist", bufs=1) as pp,
            tc.tile_pool(name="dramp", bufs=1, space="DRAM") as dp,
            tc.tile_pool(name="psacc", bufs=1, space="PSUM") as pacc,
            tc.tile_pool(name="pstr", bufs=2, space="PSUM") as ptr,
        ):
            # ---- constant loads (HWDGE; index tables first, queues split) ----
            bidx_sb = cp.tile([128, max(b_chunks, 1) * 8], I16)
            nc.scalar.dma_start(bidx_sb[:], bidx.ap())
            fgidx_sb = cp.tile([128, n_fgcols], I16)
            nc.scalar.dma_start(fgidx_sb[:], fgidx.ap())
            s1_sb = cp.tile([128, TC], F32)
            nc.sync.dma_start(s1_sb[:], s1.ap())
            s2_sb = cp.tile([128, TC], F32)
            nc.sync.dma_start(s2_sb[:], s2.ap())
            xT_sb = cp.tile([128, BL], F32)
            nc.sync.dma_start(xT_sb[:], xT.ap())
            prior_sb = cp.tile([128, K], F32)
            nc.scalar.dma_start(prior_sb[:], prior.ap())
            ident = cp.tile([128, 128], F32)
            make_identity(nc, ident[:])
            nc.gpsimd.load_library(library_config.mlp)

            # ---- truth table -> DRAM (fp8) ----
            truth_dram = dp.tile([TC * 128, BL], F8)
            td = truth_dram[:].rearrange("(c p) b -> p c b", p=128)
            truth_pools = (tc.tile_pool(name="xgp", bufs=2),
                           tc.tile_pool(name="truthp", bufs=2))
            xgp = truth_pools[0].__enter__()
            trp = truth_pools[1].__enter__()

            def compare(dst, src, c):
                if fam[c] == 0:
                    nc.vector.tensor_scalar(
                        dst, src, s1_sb[:, c:c + 1], s2_sb[:, c:c + 1],
                        OP.is_le, op1=OP.is_equal)
                else:
                    nc.vector.tensor_scalar(
                        dst, src, s1_sb[:, c:c + 1], s2_sb[:, c:c + 1],
                        OP.is_equal, op1=OP.mult)

            xg_sem = [nc.alloc_semaphore("xg_dma0"),
                      nc.alloc_semaphore("xg_dma1")]
            xg_count = [0, 0]
            n_xg = (b_chunks + XG_G - 1) // XG_G if stage >= 2 else 0

            def xg_prep(g):
                nch = min(XG_G, b_chunks - g * XG_G)
                xg = xgp.tile([128, XG_G, BL], F32, name="xgB")
                qn = g % 2
                nc.gpsimd.dma_gather(
                    xg[:, 0:nch, :], xT.ap(),
                    bidx_sb[:, g * XG_G * 8:g * XG_G * 8 + nch * 8],
                    nch * 128, nch * 128, BL,
                    prepare_only=True, sem=xg_sem[qn], queue_num=qn,
                )
                nc.gpsimd.trigger_dma(count=None, queue_num=qn)
                xg_count[qn] += 1
                return xg, nch, qn, xg_count[qn]

            xg_pend = [xg_prep(g) for g in range(min(2, n_xg))]
            n_ab = n_cmp + n_eq
            SG = 2
            for g in range((n_ab + SG - 1) // SG if stage >= 2 else 0):
                nch = min(SG, n_ab - g * SG)
                tr = trp.tile([128, SG, BL], F8)
                for i in range(nch):
                    compare(tr[:, i, :], xT_sb[:], g * SG + i)
                eng = nc.sync if g % 2 == 0 else nc.scalar
                eng.dma_start(td[:, g * SG:g * SG + nch, :],
                              tr[:, 0:nch, :])
            for g in range(n_xg):
                xg, nch, qn, cnt = xg_pend[g]
                if g + 2 < n_xg:
                    xg_pend.append(xg_prep(g + 2))
                nc.vector.wait_ge(xg_sem[qn], 16 * cnt)
                tr = trp.tile([128, XG_G, BL], F8)
                for i in range(nch):
                    compare(tr[:, i, :], xg[:, i, :], n_ab + g * XG_G + i)
                eng = nc.sync if g % 2 == 0 else nc.scalar
                eng.dma_start(
                    td[:, n_ab + g * XG_G:n_ab + g * XG_G + nch, :],
                    tr[:, 0:nch, :])
            truth_pools[1].__exit__(None, None, None)
            truth_pools[0].__exit__(None, None, None)

            # ---- prep: per-rule [delta | base_lo | base_hi] f16, wide ----
            rhs = pp.tile([128, RCC, W], F16)  # stationary per chunk
            with (
                tc.tile_pool(name="prepbig", bufs=1) as pb,
                tc.tile_pool(name="prepsm", bufs=1) as ps,
            ):
                rmp_sb = pb.tile([128, RCC * KP], F32)
                nc.scalar.dma_start(rmp_sb[:], rmp.ap())
                rmp3 = rmp_sb[:].rearrange("p (c k) -> p c k", k=KP)
                rmp_om = ps.tile([128, RCC], F32)
                nc.vector.tensor_copy(rmp_om[:], rmp3[:, :, K])
                nc.scalar.activation(rmp_sb[:], rmp_sb[:], AF.Exp)
                e3 = rmp3
                z = ps.tile([128, RCC], F32)
                nc.vector.tensor_reduce(z[:], e3, AX.X, OP.add)
                lnz = ps.tile([128, RCC], F32)
                nc.scalar.activation(lnz[:], z[:], AF.Ln)
                base = ps.tile([128, RCC], F32)
                nc.vector.tensor_tensor(base[:], rmp_om[:], lnz[:], OP.subtract)
                bh = ps.tile([128, RCC], F16)
                nc.vector.tensor_copy(bh[:], base[:])
                nc.vector.tensor_tensor(rhs[:, :, K], base[:], bh[:], OP.subtract)
                nc.vector.tensor_copy(rhs[:, :, K + 1], bh[:])
                rec = ps.tile([128, RCC], F32)
                nc.vector.reciprocal(rec[:], e3[:, :, K])
                r3 = e3[:, :, 0:K]
                nc.vector.tensor_tensor(
                    r3, r3, rec[:].unsqueeze(2).to_broadcast([128, RCC, K]),
                    OP.mult)
                nc.scalar.activation(rhs[:, :, 0:K], r3, AF.Ln, bias=1.0)

            gath_pools = (tc.tile_pool(name="ggp", bufs=6),
                          tc.tile_pool(name="tqp", bufs=3))
            ggp = gath_pools[0].__enter__()
            tqp = gath_pools[1].__enter__()
            # ---- fired + mass matmul ----
            # Rule slot s == truth table row s: the anchor literal of the
            # rule at slot s IS row s, so the anchor stream is a sequential
            # HWDGE read of the table (no SWDGE descriptors).  Only the two
            # non-anchor literals are gathered (2/3 of the descriptors).
            # Unmatched rules live in `n_extra` trailing chunks with all
            # three literals gathered.
            q_ps = []
            for h in range(NH):
                t = pacc.tile([W, 512], F32, name=f"qps{h}")
                nc.vector.memset(t[:], 0.0)
                q_ps.append(t)

            RCC_T = TC                  # anchored rule chunks
            n_gath = (RCC_T + FG - 1) // FG
            last_rc = RCC_T + n_extra - 1
            fg_sem = [nc.alloc_semaphore("fg_dma0"),
                      nc.alloc_semaphore("fg_dma1")]
            fg_count = [0, 0]
            truth_i16 = truth_dram[:].bitcast(I16)
            for fg in range(n_gath if stage >= 3 else 0):
                qn = (fg // 2) % 2
                nch = min(FG, RCC_T - fg * FG)
                pfx = TC * 128 if prefixes is None else prefixes[fg]
                gg = ggp.tile([128, 2 * FG, BL // 2], I16, name="gg")
                nc.gpsimd.dma_gather(
                    gg[:, 0:2 * nch, :], truth_dram[0:pfx, :].bitcast(I16),
                    fgidx_sb[:, fg * FW_COLS:fg * FW_COLS + 2 * nch * 8],
                    2 * nch * 128, 2 * nch * 128, BL // 2,
                    prepare_only=True, sem=fg_sem[qn], queue_num=qn,
                )
                nc.gpsimd.trigger_dma(count=None, queue_num=qn)
                fg_count[qn] += 1
                # sequential anchor block = the truth chunks themselves
                tseq = tqp.tile([128, FG, BL // 2], I16, name="tseq")
                nc.scalar.dma_start(
                    tseq[:, 0:nch, :],
                    truth_i16[fg * FG * 128:(fg * FG + nch) * 128, :]
                    .rearrange("(c p) b -> p c b", p=128))
                nc.vector.wait_ge(fg_sem[qn], 16 * fg_count[qn])
                nc.vector.tensor_tensor(
                    gg[:, 0:nch, :], gg[:, 0:nch, :],
                    gg[:, nch:2 * nch, :], OP.bitwise_and)
                nc.vector.tensor_tensor(
                    gg[:, 0:nch, :], gg[:, 0:nch, :], tseq[:, 0:nch, :],
                    OP.bitwise_and)
                fired = gg[:, 0:FG, :].bitcast(F8)  # [128, FG, BL]
                for i in range(nch if stage >= 4 else 0):
                    rc = fg * FG + i
                    for h in range(NH):
                        nc.tensor.matmul(
                            q_ps[h][:],
                            lhsT=rhs[:, rc, :],
                            rhs=fired[:, i, h * 512:(h + 1) * 512],
                            start=False, stop=(rc == last_rc),
                            skip_group_check=True,
                        )
            # extra chunks: fully gathered (3 streams)
            for e in range(n_extra if stage >= 3 else 0):
                qn = e % 2
                base = n_gath * FW_COLS + e * 24
                gg = ggp.tile([128, 3, BL // 2], I16, name="gge")
                nc.gpsimd.dma_gather(
                    gg[:], truth_dram[:].bitcast(I16),
                    fgidx_sb[:, base:base + 24],
                    3 * 128, 3 * 128, BL // 2,
                    prepare_only=True, sem=fg_sem[qn], queue_num=qn,
                )
                nc.gpsimd.trigger_dma(count=None, queue_num=qn)
                fg_count[qn] += 1
                nc.vector.wait_ge(fg_sem[qn], 16 * fg_count[qn])
                nc.vector.tensor_tensor(
                    gg[:, 0:1, :], gg[:, 0:1, :], gg[:, 1:2, :],
                    OP.bitwise_and)
                nc.vector.tensor_tensor(
                    gg[:, 0:1, :], gg[:, 0:1, :], gg[:, 2:3, :],
                    OP.bitwise_and)
                fired = gg[:, 0:1, :].bitcast(F8)
                rc = RCC_T + e
                for h in range(NH):
                    nc.tensor.matmul(
                        q_ps[h][:],
                        lhsT=rhs[:, rc, :],
                        rhs=fired[:, 0, h * 512:(h + 1) * 512],
                        start=False, stop=(rc == last_rc),
                        skip_group_check=True,
                    )

            gath_pools[1].__exit__(None, None, None)
            gath_pools[0].__exit__(None, None, None)
            ep_pool = tc.tile_pool(name="ep", bufs=1)
            ep = ep_pool.__enter__()
            # ---- transpose partials, pairwise log-space ReduceScatter ----
            qlog = ep.tile([W, BL], F32, name="qlog")
            for h in range(NH):
                nc.vector.tensor_copy(qlog[:, h * 512:(h + 1) * 512],
                                      q_ps[h][:])
            widef = ep.tile([128, NB, W], F32, name="widef")
            for g in range(NB):
                tp = ptr.tile([128, W], F32)
                nc.tensor.transpose(
                    tp[:], qlog[:, g * 128:(g + 1) * 128], ident[0:W, 0:W])
                nc.vector.tensor_copy(widef[:, g, :], tp[:])
            nc.sync.dma_start(ccin.ap(), widef[:].rearrange("p g w -> p (g w)"))
            nc.gpsimd.collective_compute(
                "ReduceScatter", OP.add,
                replica_groups=[[2 * i, 2 * i + 1] for i in range(BS)],
                ins=[ccin.ap().opt()], outs=[ccout.ap().opt()],
            )
            wide = ep.tile([64, NB, W], F32, name="wide")
            nc.scalar.dma_start(wide[:].rearrange("p g w -> p (g w)"),
                                ccout.ap())

            # ---- epilogue (wide, half rows per core) ----
            logw = ep.tile([64, NB], F32, name="logw")
            nc.vector.tensor_tensor(
                logw[:], wide[:, :, K], wide[:, :, K + 1], OP.add)
            logq = ep.tile([64, NB, K], F32, name="logq")
            nc.vector.tensor_tensor(
                logq[:], wide[:, :, 0:K],
                logw[:].unsqueeze(2).to_broadcast([64, NB, K]), OP.add)
            q = logq
            nc.scalar.activation(q[:], logq[:], AF.Exp)
            wv = ep.tile([64, NB], F32, name="wv")
            nc.scalar.activation(wv[:], logw[:], AF.Exp)
            belief = q
            nc.vector.tensor_tensor(
                belief[:], q[:],
                wv[:].unsqueeze(2).to_broadcast([64, NB, K]), OP.subtract)
            nc.vector.tensor_scalar(belief[:], belief[:], 0.0, None, OP.max)
            bsum = ep.tile([64, NB], F32, name="bsum")
            nc.vector.tensor_reduce(bsum[:], belief[:], AX.X, OP.add)
            tsum = ep.tile([64, NB], F32, name="tsum")
            nc.vector.tensor_tensor(tsum[:], bsum[:], wv[:], OP.add)
            total = ep.tile([64, NB], F32, name="total")
            nc.vector.tensor_scalar(total[:], tsum[:], EPS, None, OP.max)
            rtot = ep.tile([64, NB], F32, name="rtot")
            nc.vector.reciprocal(rtot[:], total[:])
            wp = ep.tile([64, NB, K], F32, name="wp")
            nc.vector.tensor_tensor(
                wp[:], wv[:].unsqueeze(2).to_broadcast([64, NB, K]),
                prior_sb[0:64, :].unsqueeze(1).to_broadcast([64, NB, K]),
                OP.mult)
            nc.vector.tensor_tensor(belief[:], belief[:], wp[:], OP.add)
            nc.vector.tensor_tensor(
                belief[:], belief[:],
                rtot[:].unsqueeze(2).to_broadcast([64, NB, K]), OP.mult)
            nc.sync.dma_start(
                out.ap().rearrange("(g p) k -> p g k", p=64), belief[:])
            ep_pool.__exit__(None, None, None)

    return nc


def _family_chunks(counts, cap_overflow=256):
    """Smallest per-feature chunk count n with total overflow <= cap."""
    hi = int(counts.max()) if counts.size else 0
    n = hi
    while n > 0 and int(np.maximum(counts - (n - 1), 0).sum()) <= cap_overflow:
        n -= 1
    return n


def host_layout(lit_feat_idx, lit_op_code):
    """Chunk-family layout: per-literal renumbering into the padded table."""
    fidx = np.asarray(lit_feat_idx, dtype=np.int64)
    op = np.asarray(lit_op_code)
    fam_lit = (op == 0).astype(np.int64)  # 1 for '=='
    cnt = np.zeros((2, F), np.int64)
    for fm in (0, 1):
        cnt[fm] = np.bincount(fidx[fam_lit == fm], minlength=F)
    n_cmp = _family_chunks(cnt[0])
    n_eq = _family_chunks(cnt[1])
    ov_cmp = int(np.maximum(cnt[0] - n_cmp, 0).sum())
    ov_eq = int(np.maximum(cnt[1] - n_eq, 0).sum())
    b_cmp = (ov_cmp + 127) // 128
    b_eq = (ov_eq + 127) // 128
    return n_cmp, n_eq, b_cmp, b_eq


def host_prep(X, lit_value, lit2rule, rule_len, rule_mass_params, prior,
              lit_feat_idx, lit_op_code):
    """Pure data-marshaling: shard X over batch and rules over parity,
    extract per-rule literal ids from lit2rule, compute per-literal (s1, s2)
    compare scalars for the chunk-family layout, pre-transpose rmp."""
    X = np.asarray(X, dtype=np.float32)
    lit_value = np.asarray(lit_value, dtype=np.float32)
    lit2rule = np.asarray(lit2rule, dtype=np.float32)
    rule_mass_params = np.asarray(rule_mass_params, dtype=np.float32)
    prior = np.asarray(prior, dtype=np.float32)
    op = np.asarray(lit_op_code)
    fidx = np.asarray(lit_feat_idx, dtype=np.int64)

    lT = lit2rule.T
    r_idx, l_idx = np.nonzero(lT)
    cnt_rl = lT[r_idx, l_idx].astype(np.int64)
    rl = np.repeat(l_idx, cnt_rl)
    assert rl.size == 3 * R, rl.size
    rule_lits = rl.reshape(R, 3).astype(np.int64)

    n_cmp, n_eq, b_cmp, b_eq = host_layout(fidx, op)
    n_ab = n_cmp + n_eq
    b_chunks = b_cmp + b_eq
    TC = n_ab + b_chunks
    LP = TC * 128

    # per-literal compare scalars in the family forms
    pred_v = np.nextafter(lit_value, -np.inf)
    s1 = np.where(op == 0, lit_value,
                  np.where(op == 1, pred_v, lit_value)).astype(np.float32)
    s2 = np.where(op == 0, 1.0,
                  np.where(op == 1, 1.0, 0.0)).astype(np.float32)

    # pad defaults per chunk family (always-false)
    fam = [0] * n_cmp + [1] * n_eq + [0] * b_cmp + [1] * b_eq
    s1n = np.empty(LP, np.float32)
    s2n = np.empty(LP, np.float32)
    for c in range(TC):
        if fam[c] == 0:
            s1n[c * 128:(c + 1) * 128] = -1.0
            s2n[c * 128:(c + 1) * 128] = 3.0
        else:
            s1n[c * 128:(c + 1) * 128] = -1.0
            s2n[c * 128:(c + 1) * 128] = 0.0

    newid = np.empty(L, np.int64)
    bfeat = np.zeros(max(b_chunks, 1) * 128, np.int16)
    ovbase = {0: 0, 1: b_cmp * 128}
    ovcount = [0, 0]
    fambase = {0: 0, 1: n_cmp}
    famcap = {0: n_cmp, 1: n_eq}
    for f in range(F):
        for fm in (0, 1):
            lits = np.nonzero((fidx == f) & ((op == 0) == bool(fm)))[0]
            for c, l in enumerate(lits):
                if c < famcap[fm]:
                    nid = (fambase[fm] + c) * 128 + f
                else:
                    slot = ovbase[fm] + ovcount[fm]
                    nid = (n_ab * 128) + slot
                    bfeat[slot] = f
                    ovcount[fm] += 1
                newid[l] = nid
                s1n[nid] = s1[l]
                s2n[nid] = s2[l]
    assert ovcount[0] <= b_cmp * 128 and ovcount[1] <= b_eq * 128
    rule_rows = newid[rule_lits]  # [R, 3] table rows

    def col128(v):
        return np.ascontiguousarray(np.asarray(v).reshape(-1, 128).T)

    def wrap16(ids):
        ids = np.asarray(ids, dtype=np.int16)
        return np.tile(ids.reshape(-1, 16).T, (8, 1))

    # bidx blocks are per x-gather instruction: XG_G chunks each, last partial
    bblocks = []
    g = 0
    while g * XG_G < b_chunks:
        nch = min(XG_G, b_chunks - g * XG_G)
        bblocks.append(wrap16(bfeat[g * XG_G * 128:(g * XG_G + nch) * 128]))
        g += 1
    if not bblocks:
        bblocks = [wrap16(bfeat)]
    shared = {
        "s1": col128(s1n), "s2": col128(s2n),
        "prior": np.ascontiguousarray(
            np.broadcast_to(prior.reshape(1, K), (128, K))),
        "bidx": np.concatenate(bblocks, axis=1),
    }

    # per rule half: match each rule to an anchor literal (slot == table
    # row), build the two gathered streams + overflow chunks in slot order
    import sys
    sys.setrecursionlimit(200000)
    n_slots = TC * 128
    RPAD = np.concatenate([np.full(K, -40.0, np.float32),
                           np.full(1, 40.0, np.float32)])

    def match_anchors(rows_h):
        owner = np.full(n_slots, -1, np.int64)

        def try_assign(r, visited):
            for l in rows_h[r]:
                if owner[l] == -1:
                    owner[l] = r
                    return True
            for l in rows_h[r]:
                li = int(l)
                if li not in visited:
                    visited.add(li)
                    if try_assign(owner[li], visited):
                        owner[li] = r
                        return True
            return False

        leftovers = [r for r in range(rows_h.shape[0])
                     if not try_assign(r, set())]
        return owner, leftovers

    halves = []
    n_extra_max = 0
    for h in range(RS):
        rows_h = rule_rows[h * RL:(h + 1) * RL]
        owner, leftovers = match_anchors(rows_h)
        n_extra_max = max(n_extra_max, (len(leftovers) + 127) // 128)
        halves.append((rows_h, owner, leftovers))
    n_extra = n_extra_max

    built = []
    for h in range(RS):
        rows_h, owner, leftovers = halves[h]
        # two non-anchor rows per slot (pads -> row 0)
        g12 = np.zeros((n_slots, 2), np.int64)
        rmp_rows = np.tile(RPAD, (TC * 128 + n_extra * 128, 1))
        rmp_h = rule_mass_params[h * RL:(h + 1) * RL]
        for s in range(n_slots):
            r = owner[s]
            if r < 0:
                continue
            rl = list(rows_h[r])
            rl.remove(s)
            g12[s] = rl
            rmp_rows[s] = rmp_h[r]
        n_gath = (TC + FG - 1) // FG
        fblocks = []
        prefixes = []
        for fg in range(n_gath):
            c0 = fg * FG
            nch = min(FG, TC - c0)
            ids = np.concatenate(
                [g12[(c0 + c) * 128:(c0 + c + 1) * 128, j]
                 for j in range(2) for c in range(nch)])
            blk16 = wrap16(ids)
            if blk16.shape[1] < FW_COLS:  # pad partial last gather's block
                blk16 = np.concatenate(
                    [blk16, np.zeros((128, FW_COLS - blk16.shape[1]),
                                     np.int16)], axis=1)
            fblocks.append(blk16)
            pfx = int(ids.max()) + 1
            prefixes.append(min((pfx + 511) // 512 * 512, TC * 128))
        # overflow chunks: 3 gathered streams
        ex_rows = np.zeros((n_extra * 128, 3), np.int64)
        for i, r in enumerate(leftovers):
            ex_rows[i] = rows_h[r]
            rmp_rows[n_slots + i] = rmp_h[r]
        for e in range(n_extra):
            blk = ex_rows[e * 128:(e + 1) * 128]
            ids = np.concatenate([blk[:, j] for j in range(3)])
            fblocks.append(wrap16(ids))
        fgidx = np.concatenate(fblocks, axis=1)
        RCC = TC + n_extra
        rmp_wide = np.ascontiguousarray(
            rmp_rows.reshape(RCC, 128, KP).transpose(1, 0, 2).reshape(128, -1))
        built.append((fgidx, rmp_wide, tuple(prefixes)))
    halves = built

    layout = (n_cmp, n_eq, b_cmp, b_eq, n_extra,
              tuple(max(built[0][2][i], built[1][2][i])
                    for i in range(len(built[0][2]))))
    in_maps = []
    for cid in range(N_CORES):
        s, h = cid // RS, cid % RS
        m = dict(shared)
        m["xT"] = np.ascontiguousarray(X[s * BL:(s + 1) * BL, :].T)
        m["fgidx"] = halves[h][0]
        m["rmp"] = halves[h][1]
        in_maps.append(m)
    return in_maps, layout


_NC_CACHE = {}


def build_nc_cached(layout):
    import os
    stage = int(os.environ.get("KSTAGE", "5"))
    key = (layout, stage)
    if key not in _NC_CACHE:
        nc = build_nc4(layout, stage=stage)
        nc.finalize()
        _NC_CACHE[key] = nc
    return _NC_CACHE[key]


def kernel(**inputs) -> np.ndarray:
    in_maps, layout = host_prep(
        inputs["X"], inputs["lit_value"], inputs["lit2rule"],
        inputs["rule_len"], inputs["rule_mass_params"], inputs["prior"],
        inputs["lit_feat_idx"], inputs["lit_op_code"],
    )
    nc = build_nc_cached(layout)
    res = run_bass_kernel_spmd(nc, in_maps, core_ids=list(range(N_CORES)))
    return assemble(res)


def assemble(res):
    """Each core holds rows with (b % 128) in its 64-partition half."""
    full = np.empty((B, K), np.float32)
    for cid in range(N_CORES):
        s, h = cid // RS, cid % RS
        o = res.results[cid]["out"].reshape(NB, 64, K)
        full.reshape(BS, NB, 128, K)[s, :, h * 64:(h + 1) * 64, :] = o
    return full
